# revision 7
# baseline (speedup 1.0000x reference)
"""Hamiltonian Neural ODE leapfrog integrator — Trainium2 Bass kernel.

Self-contained: takes full unsharded inputs, shards batch across 8 NeuronCores
(pure data parallel, no collectives), runs 10 leapfrog steps fully on-chip,
returns the full output.

Algorithm (per core, B_loc = 8192, transposed resident layout [dim, batch]):
  state qT~[128,8192] f32, PT~[128,8192] f32 where P = dt*p, both kept in SBUF
  per step t:
    z_c   = W1_c^T @ qT~            (TensorE, fp32r, 4 hidden chunks of 128)
    h_c   = tanh(z_c + beta[t,c])   (ScalarE LUT, per-partition bias, -> f16)
    s_c   = h_c * h_c               (VectorE f16 2x mode)
    g    += A_c @ s_c               (TensorE f16, PSUM accumulate over c)
    qT~  += PT~   (old P)           (VectorE)
    PT~  += g                       (VectorE)
  the "1 - h^2" constant and leapfrog drift are folded into precomputed
  per-step tanh biases; one final per-partition correction restores q,p.
"""
import os
import numpy as np

DT = np.float32(0.1)
STEPS = 10
B = 65536
ND = 128          # q/p dim
HID = 512
N_CORES = 8
BL = B // N_CORES  # 8192 per core
SUP = 1024         # supertile (batch cols per pipeline tile)
NJ = BL // SUP     # 8

_RUNNERS = {}      # steps -> (callable, meta)


# ---------------------------------------------------------------------------
# workarounds: this container's walrus rejects >1 sem wait per instruction
# ---------------------------------------------------------------------------
def _patch_tile_drain(tile_mod, mybir):
    if getattr(tile_mod.TileContext, "_ham_drain_patched", False):
        return

    def _drain_and_barrier(self, tick_clock, wait_clock):
        from concourse.vector_clock import ScopedClock
        nc = self.nc
        probe = nc.sync.nop(nofuse=True)
        wait_clock.add_sem_waits(
            probe.ins, ScopedClock({None: tick_clock.global_clock})
        )
        si = probe.ins.sync_info
        waits = list(si.on_wait) if (si and si.on_wait) else []
        upds = list(si.on_update) if (si and si.on_update) else []
        probe.ins.sync_info = mybir.SyncInfo(on_wait=waits[:1], on_update=upds)
        for i in range(1, len(waits)):
            extra = nc.sync.nop(nofuse=True)
            extra.ins.sync_info = mybir.SyncInfo(
                on_wait=waits[i : i + 1], on_update=[]
            )
        nc.sync.drain()
        nc.all_engine_barrier()
        assert self.sems is not None
        popped = nc._tile_sem_poison_stack.pop()
        assert popped is self._sem_poison
        nc.clear_and_free_semaphores(list(self.sems.allocated().values()))
        nc.all_engine_barrier()

    tile_mod.TileContext._drain_and_barrier = _drain_and_barrier
    tile_mod.TileContext._ham_drain_patched = True


def _split_multi_waits(nc, mybir, maxw=1):
    """Move extra sem waits onto NoOp carriers inserted before the instruction
    in the same basic block (same engine stream => ordering preserved)."""
    for f in nc.m.functions:
        for bb in f.blocks:
            out = []
            changed = False
            for ins in bb.instructions:
                si = ins.sync_info
                waits = list(si.on_wait) if (si and si.on_wait) else []
                if len(waits) > maxw:
                    movable = [w for w in waits if w.wait_reg is None]
                    pinned = [w for w in waits if w.wait_reg is not None]
                    keep_n = max(0, maxw - len(pinned))
                    keep = pinned + movable[: keep_n]
                    extra = movable[keep_n:]
                    for k, w in enumerate(extra):
                        nop = mybir.InstNoOp(
                            name=f"{ins.name}-xw{k}", engine=ins.engine,
                            ins=[], outs=[],
                        )
                        nop.sync_info = mybir.SyncInfo(on_wait=[w], on_update=[])
                        nc.register_instruction(nop)
                        out.append(nop)
                    ins.sync_info = mybir.SyncInfo(
                        on_wait=keep,
                        on_update=list(si.on_update) if si.on_update else [],
                    )
                    changed = True
                out.append(ins)
            if changed:
                bb.instructions = out


# ---------------------------------------------------------------------------
# bass program
# ---------------------------------------------------------------------------
def build_nc(steps=STEPS):
    import concourse.bass as bass
    import concourse.mybir as mybir
    import concourse.tile as tile
    from contextlib import ExitStack

    _patch_tile_drain(tile, mybir)

    f32 = mybir.dt.float32
    f32r = mybir.dt.float32r
    f16 = mybir.dt.float16
    AF = mybir.ActivationFunctionType
    ALU = mybir.AluOpType

    nc = bass.Bass(trn_type="TRN2", target_bir_lowering=False, debug=False)

    qT_d = nc.dram_tensor("qT", [ND, BL], f32r, kind="ExternalInput").ap()
    PT_d = nc.dram_tensor("PT", [ND, BL], f32, kind="ExternalInput").ap()
    w1_d = nc.dram_tensor("w1f", [ND, HID], f32r, kind="ExternalInput").ap()
    aw_d = nc.dram_tensor("awf", [ND, HID], f16, kind="ExternalInput").ap()
    bi_d = nc.dram_tensor("bias", [ND, 4 * max(steps, 1)], f32, kind="ExternalInput").ap()
    co_d = nc.dram_tensor("corr", [ND, 2], f32, kind="ExternalInput").ap()
    qo_d = nc.dram_tensor("qout", [ND, BL], f32, kind="ExternalOutput").ap()
    po_d = nc.dram_tensor("pout", [ND, BL], f32, kind="ExternalOutput").ap()

    with tile.TileContext(nc) as tc:
        with ExitStack() as ctx:
            wpool = ctx.enter_context(tc.tile_pool(name="w", bufs=1))
            state = ctx.enter_context(tc.tile_pool(name="st", bufs=1))
            zpool = ctx.enter_context(tc.tile_pool(name="z", bufs=2, space="PSUM"))
            gpool = ctx.enter_context(tc.tile_pool(name="g", bufs=2, space="PSUM"))
            hpool = ctx.enter_context(tc.tile_pool(name="h", bufs=6))
            spool = ctx.enter_context(tc.tile_pool(name="s", bufs=6))
            opool = ctx.enter_context(tc.tile_pool(name="o", bufs=4))

            w1sb = wpool.tile([ND, HID], f32r)
            awsb = wpool.tile([ND, HID], f16)
            bisb = wpool.tile([ND, 4 * max(steps, 1)], f32)
            cosb = wpool.tile([ND, 2], f32)
            nc.gpsimd.dma_start(w1sb[:], w1_d[:])
            nc.gpsimd.dma_start(awsb[:], aw_d[:])
            nc.gpsimd.dma_start(bisb[:], bi_d[:])
            nc.gpsimd.dma_start(cosb[:], co_d[:])

            qT = state.tile([ND, BL], f32r)
            PT = state.tile([ND, BL], f32)
            for j in range(NJ):
                jsl = bass.ts(j, SUP)
                nc.gpsimd.dma_start(qT[:, jsl], qT_d[:, jsl])
                nc.gpsimd.dma_start(PT[:, jsl], PT_d[:, jsl])

            w1r = w1sb[:]
            qTr = qT[:]

            for t in range(steps):
                for j in range(NJ):
                    jsl = bass.ts(j, SUP)
                    ss = []
                    for c in range(4):
                        z = zpool.tile([ND, SUP], f32)
                        for hf in range(2):
                            nc.tensor.matmul(
                                z[:, bass.ts(hf, 512)],
                                lhsT=w1r[:, bass.ts(c, 128)],
                                rhs=qTr[:, bass.ds(j * SUP + hf * 512, 512)],
                                start=True, stop=True,
                            )
                        h = hpool.tile([ND, SUP], f16)
                        nc.scalar.activation(
                            h[:], z[:], AF.Tanh,
                            bias=bisb[:, bass.ds(t * 4 + c, 1)], scale=1.0,
                        )
                        s = spool.tile([ND, SUP], f16)
                        nc.vector.tensor_tensor(s[:], h[:], h[:], ALU.mult)
                        ss.append(s)
                    g = gpool.tile([ND, SUP], f32)
                    for c in range(4):
                        for hf in range(2):
                            nc.tensor.matmul(
                                g[:, bass.ts(hf, 512)],
                                lhsT=awsb[:, bass.ts(c, 128)],
                                rhs=ss[c][:, bass.ts(hf, 512)],
                                start=(c == 0), stop=(c == 3),
                            )
                    # q += P_old must read PT before PT += g writes it
                    nc.vector.tensor_tensor(
                        qT[:, jsl], qT[:, jsl].bitcast(f32), PT[:, jsl], ALU.add
                    )
                    nc.vector.tensor_tensor(PT[:, jsl], PT[:, jsl], g[:], ALU.add)

            inv_dt = float(1.0 / DT)
            for j in range(NJ):
                jsl = bass.ts(j, SUP)
                qo = opool.tile([ND, SUP], f32)
                nc.vector.tensor_scalar(
                    qo[:], qT[:, jsl].bitcast(f32), cosb[:, bass.ds(0, 1)], None,
                    ALU.subtract,
                )
                nc.gpsimd.dma_start(qo_d[:, jsl], qo[:])
                po = opool.tile([ND, SUP], f32)
                nc.vector.tensor_scalar(
                    po[:], PT[:, jsl], cosb[:, bass.ds(1, 1)], inv_dt,
                    ALU.subtract, ALU.mult,
                )
                nc.gpsimd.dma_start(po_d[:, jsl], po[:])

    _split_multi_waits(nc, mybir)
    return nc


# ---------------------------------------------------------------------------
# runner (replicates bass2jax.run_bass_via_pjrt with a cached jit)
# ---------------------------------------------------------------------------
def _make_runner(steps=STEPS):
    import jax
    import concourse.mybir as mybir
    from concourse import bass2jax
    from concourse.bass2jax import _bass_exec_p, partition_id_tensor
    from jax.sharding import Mesh, PartitionSpec
    from jax.experimental.shard_map import shard_map

    bass2jax.install_neuronx_cc_hook()
    nc = build_nc(steps)

    in_names, out_names, out_avals = [], [], []
    partition_name = nc.partition_id_tensor.name if nc.partition_id_tensor else None
    for alloc in nc.m.functions[0].allocations:
        if not isinstance(alloc, mybir.MemoryLocationSet):
            continue
        name = alloc.memorylocations[0].name
        if alloc.kind == "ExternalInput":
            if name != partition_name:
                in_names.append(name)
        elif alloc.kind == "ExternalOutput":
            out_names.append(name)
            out_avals.append(
                jax.core.ShapedArray(tuple(alloc.tensor_shape), mybir.dt.np(alloc.dtype))
            )
    n_params = len(in_names)
    n_outs = len(out_names)
    all_in = in_names + out_names + ([partition_name] if partition_name else [])

    def _body(*args):
        operands = list(args)
        if partition_name is not None:
            operands.append(partition_id_tensor())
        return tuple(
            _bass_exec_p.bind(
                *operands,
                out_avals=tuple(out_avals), in_names=tuple(all_in),
                out_names=tuple(out_names), lowering_input_output_aliases=(),
                sim_require_finite=True, sim_require_nnan=True, nc=nc,
            )
        )

    devices = jax.devices()[:N_CORES]
    mesh = Mesh(np.asarray(devices), ("core",))
    fn = jax.jit(
        shard_map(
            _body, mesh=mesh,
            in_specs=(PartitionSpec("core"),) * (n_params + n_outs),
            out_specs=(PartitionSpec("core"),) * n_outs,
            check_rep=False,
        ),
        keep_unused=True,
    )

    def run(per_core_maps):
        concat_in = [
            np.concatenate([per_core_maps[c][n] for c in range(N_CORES)], axis=0)
            for n in in_names
        ]
        zeros = [
            np.zeros((N_CORES * a.shape[0], *a.shape[1:]), a.dtype) for a in out_avals
        ]
        outs = fn(*concat_in, *zeros)
        return [
            {
                name: np.asarray(outs[i]).reshape(N_CORES, *out_avals[i].shape)[c]
                for i, name in enumerate(out_names)
            }
            for c in range(N_CORES)
        ]

    run.jit_fn = fn
    run.in_names = in_names
    run.out_names = out_names
    run.out_avals = out_avals
    run.n_params = n_params
    return run


def get_runner(steps=STEPS):
    if steps not in _RUNNERS:
        _RUNNERS[steps] = _make_runner(steps)
    return _RUNNERS[steps]


# ---------------------------------------------------------------------------
# host prep + entry point
# ---------------------------------------------------------------------------
def _prep(x, W1, b1, W2, b2, steps=STEPS):
    x = np.ascontiguousarray(np.asarray(x, dtype=np.float32))
    W1 = np.asarray(W1, dtype=np.float32)
    b1 = np.asarray(b1, dtype=np.float32)
    W2 = np.asarray(W2, dtype=np.float32)

    dt2 = DT * DT
    A = dt2 * (W2[:, 0][:, None] * W1.T)           # [512,128]
    CC = dt2 * (W1 @ W2[:, 0])                     # [128]
    W1tCC = W1.T @ CC                              # [512]

    awf = np.zeros((ND, HID), np.float16)          # [p, c*128+k] = A[c*128+p, k]
    w1f = np.ascontiguousarray(W1)                 # [k, h'] direct
    for c in range(4):
        awf[:, c * 128:(c + 1) * 128] = A[c * 128:(c + 1) * 128, :].astype(np.float16)

    nb = 4 * max(steps, 1)
    bias = np.zeros((ND, nb), np.float32)
    for t in range(steps):
        drift = t * (t - 1) / 2.0
        beta = b1 - W1tCC * np.float32(drift)      # [512]
        for c in range(4):
            bias[:, t * 4 + c] = beta[c * 128:(c + 1) * 128]

    nstep = steps
    corr = np.zeros((ND, 2), np.float32)
    corr[:, 0] = (nstep * (nstep - 1) // 2) * CC
    corr[:, 1] = nstep * CC

    maps = []
    for i in range(N_CORES):
        rows = slice(i * BL, (i + 1) * BL)
        maps.append({
            "qT": np.ascontiguousarray(x[rows, :ND].T),
            "PT": np.ascontiguousarray((DT * x[rows, ND:]).T),
            "w1f": w1f,
            "awf": awf,
            "bias": bias,
            "corr": corr,
        })
    return maps


def kernel(x, W1, b1, W2, b2):
    steps = STEPS
    maps = _prep(x, W1, b1, W2, b2, steps)
    run = get_runner(steps)
    res = run(maps)
    out = np.empty((B, 2 * ND), np.float32)
    for i in range(N_CORES):
        rows = slice(i * BL, (i + 1) * BL)
        out[rows, :ND] = res[i]["qout"].T
        out[rows, ND:] = res[i]["pout"].T
    return out
